# revision 1
# baseline (speedup 1.0000x reference)
"""Trainium2 Bass kernel for nn_CPCircuitLayer (embedding_lookup).

Math: A = X @ W_seq^T  [S,R];  Bm = X^T @ W_hid^T  [H,R]
      out[b, n] = dot(A[b, idx_s[n]], Bm[b, idx_h[n]]),  out -> [B, S, H]

Sharding (8 cores, no collectives): core c handles batch b = c//4 and the
quarter q = c%4 of the N = S*H index list (J = N/4 indices). Both factor
tables are computed redundantly per batch group from the full X[b].

Per-core device pipeline:
  1. Load X[b] (bf16) + transposed copy via HWDGE transpose-DMA.
  2. PE matmuls (bf16 in, f32 psum): A^T and Bm^T [32, 1024].
  3. Repack to per-lane split-R tables: partition p holds columns
     2*(p%16), 2*(p%16)+1 of the factor interleaved ([128, 1024, 2] f32),
     via a DRAM bounce + 8x partition-group broadcast load.
  4. ap_gather (GPSIMD FIFO): each 16-partition group streams its own
     indices; one instruction gathers NIdx rows x 8 groups.
  5. DVE mul + pair-sum, then PE block-indicator matmul reduces the 16
     lanes x 2 of each group -> psum [8, n] -> out.
"""

import numpy as np
import ml_dtypes
from contextlib import ExitStack

import concourse.bass as bass
import concourse.mybir as mybir
import concourse.tile as tile
from concourse import bacc

B, S, H, R = 2, 1024, 1024, 32
N = S * H
NCORES = 8
J = N // 4            # outputs per core (one batch, quarter of N) = 262144
JG = J // 8           # outputs per 16-partition group = 32768
NIdx = 2048           # indices per group per ap_gather instruction
RNDS = JG // NIdx     # 16 gather rounds per table
GRP_D = 2             # table f32 per lane (R = 16 lanes * 2)
SKIP_GATHER = False   # timing experiment: drop ap_gather instructions

F32 = mybir.dt.float32
BF16 = mybir.dt.bfloat16
I16 = mybir.dt.int16


def _build(reps: int = 1):
    nc = bacc.Bacc()
    x = nc.declare_dram_parameter("x", [S, H], BF16, False)
    wseq_t = nc.declare_dram_parameter("wseq_t", [H, R], BF16, False)
    whid_t = nc.declare_dram_parameter("whid_t", [S, R], BF16, False)
    # per-group index streams, wrapped: group g's jj-th index lives at
    # [16*g + jj%16, jj//16]
    idx_s = nc.declare_dram_parameter("idx_s", [128, 2 * JG // 16], I16, False)
    idx_h = nc.declare_dram_parameter("idx_h", [128, 2 * JG // 16], I16, False)
    ind_in = nc.declare_dram_parameter("ind", [128, 8], F32, False)
    out = nc.declare_dram_parameter("out", [8, JG], F32, True)
    ta_dram = nc.dram_tensor("ta", [R, S], F32)   # A^T bounce
    tb_dram = nc.dram_tensor("tb", [R, H], F32)   # Bm^T bounce

    with tile.TileContext(nc) as tc, ExitStack() as ctx:
        base = ctx.enter_context(tc.tile_pool(name="base", bufs=1))
        psum = ctx.enter_context(tc.tile_pool(name="psum", bufs=2, space="PSUM"))
        rpsum = ctx.enter_context(tc.tile_pool(name="rpsum", bufs=1, space="PSUM"))
        stage = ctx.enter_context(tc.tile_pool(name="stage", bufs=2))
        gap = ctx.enter_context(tc.tile_pool(name="gap", bufs=2))
        gbp = ctx.enter_context(tc.tile_pool(name="gbp", bufs=2))
        prodp = ctx.enter_context(tc.tile_pool(name="prodp", bufs=2))
        otp = ctx.enter_context(tc.tile_pool(name="otp", bufs=1))

        # --- loads -------------------------------------------------------
        x_sb = base.tile([128, 8, H], BF16)       # X[s,h]: p=s%128, k=s//128
        xt_sb = base.tile([128, 8, S], BF16)      # X^T[h,s]: p=h%128, k=h//128
        ws_sb = base.tile([128, 8, R], BF16)      # W_seq^T rows (h-major)
        wh_sb = base.tile([128, 8, R], BF16)      # W_hid^T rows (s-major)
        isb_s = base.tile([128, 2 * JG // 16], I16)
        isb_h = base.tile([128, 2 * JG // 16], I16)
        ind_sb = base.tile([128, 8], F32)         # block indicator for reduce
        ta_sb = base.tile([128, 2 * S], F32)
        tb_sb = base.tile([128, 2 * H], F32)

        nc.sync.dma_start(
            out=x_sb[:],
            in_=bass.AP(tensor=x[:].tensor, offset=0,
                        ap=[[H, 128], [128 * H, 8], [1, H]]),
        )
        for k in range(8):
            nc.sync.dma_start_transpose(
                out=xt_sb[:, k, :], in_=x[:, 128 * k:128 * (k + 1)]
            )
        nc.sync.dma_start(
            out=ws_sb[:],
            in_=bass.AP(tensor=wseq_t[:].tensor, offset=0,
                        ap=[[R, 128], [128 * R, 8], [1, R]]),
        )
        nc.sync.dma_start(
            out=wh_sb[:],
            in_=bass.AP(tensor=whid_t[:].tensor, offset=0,
                        ap=[[R, 128], [128 * R, 8], [1, R]]),
        )
        nc.sync.dma_start(out=isb_s[:], in_=idx_s[:])
        nc.sync.dma_start(out=isb_h[:], in_=idx_h[:])

        nc.sync.dma_start(out=ind_sb[:], in_=ind_in[:])

        for _ in range(reps):
            _body(nc, psum, rpsum, stage, gap, gbp, prodp, otp,
                  x_sb, xt_sb, ws_sb, wh_sb, isb_s, isb_h, ind_sb,
                  ta_sb, tb_sb, ta_dram, tb_dram, out)
    nc.compile()
    return nc


def _body(nc, psum, rpsum, stage, gap, gbp, prodp, otp,
          x_sb, xt_sb, ws_sb, wh_sb, isb_s, isb_h, ind_sb,
          ta_sb, tb_sb, ta_dram, tb_dram, out):
    # --- factor transposes on PE: F^T [32, 1024] ------------------------
    # A^T[r, s] = sum_h Wseq^T[h, r] X^T[h, s]; Bm^T[r, h] = sum_s ...
    for (tdram, lhs_w, rhs_x) in ((ta_dram, ws_sb, xt_sb),
                                  (tb_dram, wh_sb, x_sb)):
        ft = stage.tile([R, S], F32, tag="ft")
        for nh in range(2):
            pt = psum.tile([R, S // 2], F32, tag="pt")
            for k in range(8):
                nc.tensor.matmul(
                    out=pt[:],
                    lhsT=lhs_w[:, k, :],
                    rhs=rhs_x[:, k, nh * 512:(nh + 1) * 512],
                    start=(k == 0), stop=(k == 7),
                )
            nc.vector.tensor_copy(out=ft[:, nh * 512:(nh + 1) * 512], in_=pt[:])
        nc.gpsimd.dma_start(out=tdram[:], in_=ft[:])

    # broadcast tables back, lane-split d=2 interleaved: partition p
    # (lane l = p%16) holds tab[p, v, d] = F^T[2l+d, v]
    for (tdram, tsb, V) in ((ta_dram, ta_sb, S), (tb_dram, tb_sb, H)):
        nc.gpsimd.dma_start(
            out=tsb[:],
            in_=bass.AP(tensor=tdram[:].tensor, offset=0,
                        ap=[[0, 8], [2 * V, 16], [1, 2 * V]]),
        )

    # --- gather + reduce ------------------------------------------------
    ot = None
    for rnd in range(RNDS):
        isl = slice(rnd * (2 * NIdx // 16), (rnd + 1) * (2 * NIdx // 16))
        ga = gap.tile([128, NIdx, GRP_D], F32, tag="ga")
        gb = gbp.tile([128, NIdx, GRP_D], F32, tag="gb")
        ga_flat = bass.AP(tensor=ga[:].tensor, offset=ga[:].offset,
                          ap=[list(ga[:].ap[0]), [1, 2 * NIdx], [1, 1]])
        gb_flat = bass.AP(tensor=gb[:].tensor, offset=gb[:].offset,
                          ap=[list(gb[:].ap[0]), [1, 2 * NIdx], [1, 1]])
        if SKIP_GATHER:
            nc.vector.memset(ga[:], 0.0)
            nc.vector.memset(gb[:], 0.0)
        else:
            nc.gpsimd.ap_gather(
                out_ap=ga_flat, in_ap=ta_sb[:], idxs_ap=isb_s[:, isl],
                channels=128, num_elems=2 * S, d=1, num_idxs=2 * NIdx,
            )
            nc.gpsimd.ap_gather(
                out_ap=gb_flat, in_ap=tb_sb[:], idxs_ap=isb_h[:, isl],
                channels=128, num_elems=2 * H, d=1, num_idxs=2 * NIdx,
            )
        prod = prodp.tile([128, NIdx, GRP_D], F32, tag="prod")
        nc.vector.tensor_mul(prod[:], ga[:], gb[:])
        p2 = prodp.tile([128, NIdx], F32, tag="p2")
        nc.vector.tensor_add(p2[:], prod[:, :, 0], prod[:, :, 1])
        # reduce 16 lanes per group via block-indicator matmul; all four
        # 512-col results land in one 4-bank psum tile -> single copy;
        # out-DMA once per two rounds
        if rnd % 2 == 0:
            ot = otp.tile([8, 2 * NIdx], F32, tag="ot")
        rp4 = rpsum.tile([8, NIdx], F32, tag="rp4")
        for t in range(NIdx // 512):
            nc.tensor.matmul(
                out=rp4[:, t * 512:(t + 1) * 512],
                lhsT=ind_sb[:],
                rhs=p2[:, t * 512:(t + 1) * 512],
                start=True, stop=True,
            )
        half = (rnd % 2) * NIdx
        nc.scalar.copy(out=ot[:, half:half + NIdx], in_=rp4[:])
        if rnd % 2 == 1:
            nc.sync.dma_start(
                out=bass.AP(tensor=out[:].tensor, offset=(rnd - 1) * NIdx,
                            ap=[[JG, 8], [1, 2 * NIdx]]),
                in_=ot[:],
            )


_nc_cache_by_reps = {}


def _get_nc(reps: int = 1):
    nc = _nc_cache_by_reps.get(reps)
    if nc is None:
        nc = _nc_cache_by_reps[reps] = _build(reps)
    return nc


class _Runner:
    """Trace/compile the SPMD executable once; reuse across calls."""

    def __init__(self, nc):
        import jax
        from jax.experimental.shard_map import shard_map
        from jax.sharding import Mesh, PartitionSpec
        import concourse.bass2jax as b2j

        b2j.install_neuronx_cc_hook()
        self.nc = nc
        part_name = (nc.partition_id_tensor.name
                     if nc.partition_id_tensor else None)
        in_names, out_names, out_avals = [], [], []
        zero_outs = []
        for alloc in nc.m.functions[0].allocations:
            if not isinstance(alloc, mybir.MemoryLocationSet):
                continue
            name = alloc.memorylocations[0].name
            if alloc.kind == "ExternalInput":
                if name != part_name:
                    in_names.append(name)
            elif alloc.kind == "ExternalOutput":
                out_names.append(name)
                shape = tuple(alloc.tensor_shape)
                dtype = mybir.dt.np(alloc.dtype)
                out_avals.append(jax.core.ShapedArray(shape, dtype))
                zero_outs.append(np.zeros(shape, dtype))
        self.in_names = list(in_names)
        self.out_names = out_names
        self.zero_outs = zero_outs
        n_params = len(in_names)
        n_outs = len(out_names)
        all_in_names = in_names + out_names
        if part_name is not None:
            all_in_names = all_in_names + [part_name]
        donate = tuple(range(n_params, n_params + n_outs))

        def _body_fn(*args):
            operands = list(args)
            if part_name is not None:
                operands.append(b2j.partition_id_tensor())
            outs = b2j._bass_exec_p.bind(
                *operands,
                out_avals=tuple(out_avals),
                in_names=tuple(all_in_names),
                out_names=tuple(out_names),
                lowering_input_output_aliases=(),
                sim_require_finite=True,
                sim_require_nnan=True,
                nc=nc,
            )
            return tuple(outs)

        devices = jax.devices()[:NCORES]
        mesh = Mesh(np.asarray(devices), ("core",))
        self.fn = jax.jit(
            shard_map(
                _body_fn, mesh=mesh,
                in_specs=(PartitionSpec("core"),) * (n_params + n_outs),
                out_specs=(PartitionSpec("core"),) * n_outs,
                check_rep=False,
            ),
            donate_argnums=donate,
            keep_unused=True,
        )

    def __call__(self, in_maps):
        concat_in = [
            np.concatenate([np.asarray(m[name]) for m in in_maps], axis=0)
            for name in self.in_names
        ]
        concat_zeros = [
            np.zeros((NCORES * z.shape[0], *z.shape[1:]), z.dtype)
            for z in self.zero_outs
        ]
        out_arrs = self.fn(*concat_in, *concat_zeros)
        return [
            {
                name: np.asarray(out_arrs[i]).reshape(NCORES, -1)[c]
                for i, name in enumerate(self.out_names)
            }
            for c in range(NCORES)
        ]


_runner_cache = {}


def _get_runner(reps: int = 1):
    r = _runner_cache.get(reps)
    if r is None:
        r = _runner_cache[reps] = _Runner(_get_nc(reps))
    return r


def _wrap_idx(v: np.ndarray) -> np.ndarray:
    """[J] -> [128, 2*JG/16] int16: group g = j // JG streams the pairs
    (v, v+1024) for its outputs, wrapped at [16*g + t%16, t//16]."""
    v = v.astype(np.int16)
    v2 = np.empty(2 * J, np.int16)
    v2[0::2] = v
    v2[1::2] = v + 1024
    w = v2.reshape(8, 2 * JG // 16, 16)   # [g, col, p16]
    w = w.transpose(0, 2, 1).reshape(128, 2 * JG // 16)
    return np.ascontiguousarray(w)


def prepare_in_maps(hidden_states, W_seq, W_hid, all_indices):
    x_bf = [np.ascontiguousarray(hidden_states[b].astype(ml_dtypes.bfloat16))
            for b in range(B)]
    ws_t = np.ascontiguousarray(W_seq.T.astype(ml_dtypes.bfloat16))
    wh_t = np.ascontiguousarray(W_hid.T.astype(ml_dtypes.bfloat16))
    idx_pairs = []
    for q in range(4):
        seg = all_indices[q * J:(q + 1) * J]
        idx_pairs.append((_wrap_idx(seg[:, 0]), _wrap_idx(seg[:, 1])))
    in_maps = []
    for c in range(NCORES):
        b, q = c // 4, c % 4
        ind = np.zeros((128, 8), np.float32)
        for g in range(8):
            ind[16 * g:16 * (g + 1), g] = 1.0
        in_maps.append({
            "x": x_bf[b],
            "wseq_t": ws_t,
            "whid_t": wh_t,
            "idx_s": idx_pairs[q][0],
            "idx_h": idx_pairs[q][1],
            "ind": ind,
        })
    return in_maps


def kernel(hidden_states, W_seq, W_hid, all_indices):
    hidden_states = np.asarray(hidden_states)
    W_seq = np.asarray(W_seq)
    W_hid = np.asarray(W_hid)
    all_indices = np.asarray(all_indices)

    runner = _get_runner()
    in_maps = prepare_in_maps(hidden_states, W_seq, W_hid, all_indices)
    results = runner(in_maps)

    out = np.empty((B, N), dtype=np.float32)
    for c in range(NCORES):
        b, q = c // 4, c % 4
        o = results[c]["out"].reshape(8, JG)
        # out[g, jj] holds output j = g*JG + jj of this core's quarter
        out[b, q * J:(q + 1) * J] = o.reshape(J)
    return out.reshape(B, S, H)



# revision 2
# speedup vs baseline: 1.6845x; 1.6845x over previous
"""Trainium2 Bass kernel for nn_CPCircuitLayer (embedding_lookup), v4.

Math: out[b, n] = dot(A[b, idx_s[n]], Bm[b, idx_h[n]]) = M_b[idx_s[n], idx_h[n]]
where M_b = A_b @ Bm_b^T is a [S, H] table, A = X W_seq^T, Bm^T = W_hid X.

Sharding (8 cores, no collectives): core c handles batch b = c//4 and the
h-slice q = c%4 (columns [256q, 256q+256)). It builds only its M slice
[1024, 256] laid out as [128 partitions, 2048]: partition p = s%128,
offset o = (s//128)*256 + h%256. Each output n is routed to the core
owning its table element.

Per-core device pipeline:
  1. PE matmuls (bf16 in, f32 psum): A^T [32,1024], B^T [32,256], then
     M = A @ B^T -> sbuf bf16 [128, 2048].
  2. local_scatter cascade: pass k serves the rank-k user of every table
     element (host-prepped per-partition dst slots, in key order). Pass 0
     scans the table; pass k>=1 scans pass (k-1)'s dst buffer, which is
     exactly the compacted list of elements with >= k users. Scan lengths
     shrink geometrically, so 12 passes (multiplicity <= 12) cost barely
     more than 3. The packed dst buffer IS the output: one DMA to DRAM
     (split across queues), host unpermutes.
  3. Outputs whose element overflowed a dst cap or has multiplicity > 12
     (never happens for uniform random indices) are computed on host.
"""

import numpy as np
import ml_dtypes
from contextlib import ExitStack

import concourse.bass as bass
import concourse.mybir as mybir
import concourse.tile as tile
from concourse import bacc

B, S, H, R = 2, 1024, 1024, 32
N = S * H
NCORES = 8
HQ = H // 4           # h-columns per core
E = 2048              # table elements per partition (1024*256/128)
DSTS = [1536, 768, 288, 96, 48, 32, 16, 16, 8, 8, 8, 8]  # per-pass dst caps
PASSES = len(DSTS)
SCANS = [E] + DSTS[:-1]                # pass k scans pass k-1's dst
DOFF = np.cumsum([0] + DSTS).tolist()  # dst offsets in packed output
IOFF = np.cumsum([0] + SCANS).tolist()  # idx offsets in packed si input
ODW = DOFF[-1]        # packed output width (2832)
SIW = IOFF[-1]        # packed int16 input width (4872)

F32 = mybir.dt.float32
BF16 = mybir.dt.bfloat16
I16 = mybir.dt.int16


def _build(reps: int = 1, compile: bool = True):
    nc = bacc.Bacc()
    x = nc.declare_dram_parameter("x", [S, H], BF16, False)
    xs = nc.declare_dram_parameter("xs", [S, HQ], BF16, False)
    wseq_t = nc.declare_dram_parameter("wseq_t", [H, R], BF16, False)
    whid_t = nc.declare_dram_parameter("whid_t", [S, R], BF16, False)
    si_all = nc.declare_dram_parameter("si_all", [128, SIW], I16, False)
    od = nc.declare_dram_parameter("od", [128, ODW], BF16, True)

    with tile.TileContext(nc) as tc, ExitStack() as ctx:
        base = ctx.enter_context(tc.tile_pool(name="base", bufs=1))
        psA = ctx.enter_context(tc.tile_pool(name="psA", bufs=2, space="PSUM"))
        psM = ctx.enter_context(tc.tile_pool(name="psM", bufs=2, space="PSUM"))
        ab = ctx.enter_context(tc.tile_pool(name="ab", bufs=2))
        mp = ctx.enter_context(tc.tile_pool(name="mp", bufs=2))
        dsp = ctx.enter_context(tc.tile_pool(name="dsp", bufs=2))

        # --- one-time loads ----------------------------------------------
        xt_sb = base.tile([128, 8, S], BF16)      # X^T[h,s]: p=h%128, k=h//128
        xs_sb = base.tile([128, 8, HQ], BF16)     # X[:, hq]: p=s%128, k=s//128
        ws_sb = base.tile([128, 8, R], BF16)      # W_seq^T rows (h-major)
        wh_sb = base.tile([128, 8, R], BF16)      # W_hid^T rows (s-major)
        si_sb = base.tile([128, SIW], I16)        # cascade idx streams

        for k in range(8):
            nc.sync.dma_start_transpose(
                out=xt_sb[:, k, :], in_=x[:, 128 * k:128 * (k + 1)]
            )
        nc.sync.dma_start(
            out=xs_sb[:],
            in_=bass.AP(tensor=xs[:].tensor, offset=0,
                        ap=[[HQ, 128], [128 * HQ, 8], [1, HQ]]),
        )
        nc.sync.dma_start(
            out=ws_sb[:],
            in_=bass.AP(tensor=wseq_t[:].tensor, offset=0,
                        ap=[[R, 128], [128 * R, 8], [1, R]]),
        )
        nc.sync.dma_start(
            out=wh_sb[:],
            in_=bass.AP(tensor=whid_t[:].tensor, offset=0,
                        ap=[[R, 128], [128 * R, 8], [1, R]]),
        )
        nc.sync.dma_start(out=si_sb[:], in_=si_all[:])

        for _ in range(reps):
            _body(nc, psA, psM, ab, mp, dsp,
                  xt_sb, xs_sb, ws_sb, wh_sb, si_sb, od)
    if compile:
        nc.compile()
    return nc


def _body(nc, psA, psM, ab, mp, dsp,
          xt_sb, xs_sb, ws_sb, wh_sb, si_sb, od):
    # --- factor matmuls: A^T [32, 1024], B^T [32, 256] -------------------
    a_t = ab.tile([R, S], BF16, tag="a_t")
    for nh in range(2):
        pa = psA.tile([R, S // 2], F32, tag="pa")
        for k in range(8):
            nc.tensor.matmul(
                out=pa[:], lhsT=ws_sb[:, k, :],
                rhs=xt_sb[:, k, nh * 512:(nh + 1) * 512],
                start=(k == 0), stop=(k == 7),
            )
        nc.vector.tensor_copy(out=a_t[:, nh * 512:(nh + 1) * 512], in_=pa[:])
    b_t = ab.tile([R, HQ], BF16, tag="b_t")
    pb = psA.tile([R, HQ], F32, tag="pb")
    for k in range(8):
        nc.tensor.matmul(
            out=pb[:], lhsT=wh_sb[:, k, :], rhs=xs_sb[:, k, :],
            start=(k == 0), stop=(k == 7),
        )
    nc.vector.tensor_copy(out=b_t[:], in_=pb[:])

    # --- M slice: [128, 2048] bf16 ---------------------------------------
    m_b = mp.tile([128, E], BF16, tag="m_b")
    for kb2 in range(4):
        pm = psM.tile([128, 2 * HQ], F32, tag="pm")
        for j in range(2):
            kb = 2 * kb2 + j
            nc.tensor.matmul(
                out=pm[:, j * HQ:(j + 1) * HQ],
                lhsT=a_t[:, kb * 128:(kb + 1) * 128], rhs=b_t[:],
                start=True, stop=True,
            )
        nc.vector.tensor_copy(out=m_b[:, kb2 * 2 * HQ:(kb2 + 1) * 2 * HQ],
                              in_=pm[:])

    # --- local_scatter cascade ------------------------------------------
    ds = dsp.tile([128, ODW], BF16, tag="ds")
    for i in range(PASSES):
        data = m_b[:] if i == 0 else ds[:, DOFF[i - 1]:DOFF[i]]
        nc.gpsimd.local_scatter(
            out_ap=ds[:, DOFF[i]:DOFF[i + 1]], data_ap=data,
            idxs_ap=si_sb[:, IOFF[i]:IOFF[i + 1]],
            channels=128, num_elems=DSTS[i], num_idxs=SCANS[i],
        )
    cut1, cut2 = 1024, 2048
    nc.scalar.dma_start(
        out=bass.AP(tensor=od[:].tensor, offset=0,
                    ap=[[ODW, 128], [1, cut1]]),
        in_=ds[:, :cut1],
    )
    nc.sync.dma_start(
        out=bass.AP(tensor=od[:].tensor, offset=cut1,
                    ap=[[ODW, 128], [1, cut2 - cut1]]),
        in_=ds[:, cut1:cut2],
    )
    nc.gpsimd.dma_start(
        out=bass.AP(tensor=od[:].tensor, offset=cut2,
                    ap=[[ODW, 128], [1, ODW - cut2]]),
        in_=ds[:, cut2:],
    )


# ---------------------------------------------------------------------------
# Host-side routing
# ---------------------------------------------------------------------------

def _group_slots(keys):
    """Per-group running index for a sorted int array."""
    n = len(keys)
    if n == 0:
        return np.zeros(0, np.int64)
    first = np.r_[True, keys[1:] != keys[:-1]]
    starts = np.flatnonzero(first)
    counts = np.diff(np.r_[starts, n])
    return np.arange(n) - np.repeat(starts, counts)


def _route_quarter(s, h, n_sel):
    """Route one quarter's outputs through the scatter cascade.

    Returns (si_all [128, SIW] i16, (n_ids, od flat positions) for
    device-served users, fallback n_ids)."""
    p = (s & 127).astype(np.int64)
    o = (((s >> 7) << 8) | (h & 255)).astype(np.int64)
    key = p * E + o
    order = np.argsort(key, kind="stable")
    ks = key[order]
    n_ord = n_sel[order]
    rank = _group_slots(ks)

    # element table (unique keys, key order)
    first = np.r_[True, ks[1:] != ks[:-1]]
    el_key = ks[first]
    el_cnt = np.diff(np.r_[np.flatnonzero(first), len(ks)])
    el_p = el_key // E
    el_o = el_key % E
    ne = len(el_key)
    u_el = np.cumsum(first) - 1           # user -> element index

    si_arr = np.full((128, SIW), -1, np.int16)
    el_slot = np.full((PASSES, ne), -1, np.int64)
    alive = np.ones(ne, bool)
    for k in range(PASSES):
        cand = alive & (el_cnt >= k + 1)
        idxs = np.flatnonzero(cand)
        slot = _group_slots(el_p[idxs])
        ovf = slot >= DSTS[k]
        if ovf.any():
            alive[idxs[ovf]] = False      # demote element's remaining users
            idxs, slot = idxs[~ovf], slot[~ovf]
        el_slot[k, idxs] = slot
        # device idx stream for pass k, indexed by data position j
        jpos = el_o[idxs] if k == 0 else el_slot[k - 1, idxs]
        si_arr[el_p[idxs], IOFF[k] + jpos] = slot.astype(np.int16)

    u_slot = np.where(rank < PASSES,
                      el_slot[np.minimum(rank, PASSES - 1), u_el], -1)
    okm = u_slot >= 0
    pos = (el_p[u_el[okm]] * ODW + np.asarray(DOFF)[rank[okm]]
           + u_slot[okm]).astype(np.int64)
    return si_arr, (n_ord[okm], pos), n_ord[~okm]


def prepare_in_maps(hidden_states, W_seq, W_hid, all_indices):
    x_bf = [np.ascontiguousarray(hidden_states[b].astype(ml_dtypes.bfloat16))
            for b in range(B)]
    ws_t = np.ascontiguousarray(W_seq.T.astype(ml_dtypes.bfloat16))
    wh_t = np.ascontiguousarray(W_hid.T.astype(ml_dtypes.bfloat16))

    s_idx = np.asarray(all_indices[:, 0], dtype=np.int64)
    h_idx = np.asarray(all_indices[:, 1], dtype=np.int64)
    qarr = h_idx >> 8

    routes = []
    for q in range(4):
        n_sel = np.flatnonzero(qarr == q)
        routes.append(_route_quarter(s_idx[n_sel], h_idx[n_sel], n_sel))

    in_maps = []
    for c in range(NCORES):
        b, q = c // 4, c % 4
        si_arr, _, _ = routes[q]
        in_maps.append({
            "x": x_bf[b],
            "xs": np.ascontiguousarray(x_bf[b][:, HQ * q:HQ * (q + 1)]),
            "wseq_t": ws_t,
            "whid_t": wh_t,
            "si_all": si_arr,
        })
    return in_maps, routes


def assemble(results, routes, hidden_states, W_seq, W_hid, all_indices):
    out = np.empty((B, N), dtype=np.float32)
    fb_cache = {}
    for c in range(NCORES):
        b, q = c // 4, c % 4
        _, (n_ids, pos), n_fb = routes[q]
        buf = np.asarray(results[c]["od"], np.float32).reshape(-1)
        out[b, n_ids] = buf[pos]
        if len(n_fb):
            # host fallback: elements that overflowed a dst cap or have
            # multiplicity > PASSES (never for uniform random indices)
            if b not in fb_cache:
                X = np.asarray(hidden_states[b], np.float32)
                A = X @ np.asarray(W_seq, np.float32).T        # [S, R]
                Bm = X.T @ np.asarray(W_hid, np.float32).T     # [H, R]
                fb_cache[b] = (A, Bm)
            A, Bm = fb_cache[b]
            si = np.asarray(all_indices[n_fb, 0], np.int64)
            hi = np.asarray(all_indices[n_fb, 1], np.int64)
            out[b, n_fb] = np.einsum("nr,nr->n", A[si], Bm[hi])
    return out.reshape(B, S, H)


# ---------------------------------------------------------------------------
# Runner (trace/compile SPMD executable once, reuse)
# ---------------------------------------------------------------------------

_nc_cache_by_reps = {}


def _get_nc(reps: int = 1):
    nc = _nc_cache_by_reps.get(reps)
    if nc is None:
        nc = _nc_cache_by_reps[reps] = _build(reps)
    return nc


class _Runner:
    """Trace/compile the SPMD executable once; reuse across calls."""

    def __init__(self, nc, donate=True):
        import jax
        from jax.experimental.shard_map import shard_map
        from jax.sharding import Mesh, PartitionSpec
        import concourse.bass2jax as b2j

        b2j.install_neuronx_cc_hook()
        self.nc = nc
        part_name = (nc.partition_id_tensor.name
                     if nc.partition_id_tensor else None)
        in_names, out_names, out_avals = [], [], []
        zero_outs = []
        for alloc in nc.m.functions[0].allocations:
            if not isinstance(alloc, mybir.MemoryLocationSet):
                continue
            name = alloc.memorylocations[0].name
            if alloc.kind == "ExternalInput":
                if name != part_name:
                    in_names.append(name)
            elif alloc.kind == "ExternalOutput":
                out_names.append(name)
                shape = tuple(alloc.tensor_shape)
                dtype = mybir.dt.np(alloc.dtype)
                out_avals.append(jax.core.ShapedArray(shape, dtype))
                zero_outs.append(np.zeros(shape, dtype))
        self.in_names = list(in_names)
        self.out_names = out_names
        self.zero_outs = zero_outs
        n_params = len(in_names)
        n_outs = len(out_names)
        all_in_names = in_names + out_names
        if part_name is not None:
            all_in_names = all_in_names + [part_name]
        donate_nums = (tuple(range(n_params, n_params + n_outs))
                       if donate else ())

        def _body_fn(*args):
            operands = list(args)
            if part_name is not None:
                operands.append(b2j.partition_id_tensor())
            outs = b2j._bass_exec_p.bind(
                *operands,
                out_avals=tuple(out_avals),
                in_names=tuple(all_in_names),
                out_names=tuple(out_names),
                lowering_input_output_aliases=(),
                sim_require_finite=True,
                sim_require_nnan=True,
                nc=nc,
            )
            return tuple(outs)

        devices = jax.devices()[:NCORES]
        mesh = Mesh(np.asarray(devices), ("core",))
        self.fn = jax.jit(
            shard_map(
                _body_fn, mesh=mesh,
                in_specs=(PartitionSpec("core"),) * (n_params + n_outs),
                out_specs=(PartitionSpec("core"),) * n_outs,
                check_rep=False,
            ),
            donate_argnums=donate_nums,
            keep_unused=True,
        )
        self.mesh = mesh

    def __call__(self, in_maps):
        concat_in = [
            np.concatenate([np.asarray(m[name]) for m in in_maps], axis=0)
            for name in self.in_names
        ]
        concat_zeros = [
            np.zeros((NCORES * z.shape[0], *z.shape[1:]), z.dtype)
            for z in self.zero_outs
        ]
        out_arrs = self.fn(*concat_in, *concat_zeros)
        return [
            {
                name: np.asarray(out_arrs[i]).reshape(
                    NCORES, *self.zero_outs[i].shape)[c]
                for i, name in enumerate(self.out_names)
            }
            for c in range(NCORES)
        ]


_runner_cache = {}


def _get_runner(reps: int = 1):
    r = _runner_cache.get(reps)
    if r is None:
        r = _runner_cache[reps] = _Runner(_get_nc(reps))
    return r


def kernel(hidden_states, W_seq, W_hid, all_indices):
    hidden_states = np.asarray(hidden_states)
    W_seq = np.asarray(W_seq)
    W_hid = np.asarray(W_hid)
    all_indices = np.asarray(all_indices)

    runner = _get_runner()
    in_maps, routes = prepare_in_maps(hidden_states, W_seq, W_hid, all_indices)
    results = runner(in_maps)
    return assemble(results, routes, hidden_states, W_seq, W_hid, all_indices)


# revision 3
# speedup vs baseline: 3.3134x; 1.9670x over previous
"""Trainium2 Bass kernel for nn_CPCircuitLayer (embedding_lookup), v4.

Math: out[b, n] = dot(A[b, idx_s[n]], Bm[b, idx_h[n]]) = M_b[idx_s[n], idx_h[n]]
where M_b = A_b @ Bm_b^T is a [S, H] table, A = X W_seq^T, Bm^T = W_hid X.

Sharding (8 cores, no collectives): core c handles batch b = c//4 and the
h-slice q = c%4 (columns [256q, 256q+256)). It builds only its M slice
[1024, 256] laid out as [128 partitions, 2048]: partition p = s%128,
offset o = (s//128)*256 + h%256. Each output n is routed to the core
owning its table element.

Per-core device pipeline:
  1. PE matmuls (bf16 in, f32 psum): A^T [32,1024], B^T [32,256], then
     M = A @ B^T -> sbuf bf16 [128, 2048].
  2. local_scatter cascade: pass k serves the rank-k user of every table
     element (host-prepped per-partition dst slots, in key order). Pass 0
     scans the table; pass k>=1 scans pass (k-1)'s dst buffer, which is
     exactly the compacted list of elements with >= k users. Scan lengths
     shrink geometrically, so 12 passes (multiplicity <= 12) cost barely
     more than 3. The packed dst buffer IS the output: one DMA to DRAM
     (split across queues), host unpermutes.
  3. Outputs whose element overflowed a dst cap or has multiplicity > 12
     (never happens for uniform random indices) are computed on host.
"""

import numpy as np
import ml_dtypes
from contextlib import ExitStack

import concourse.bass as bass
import concourse.mybir as mybir
import concourse.tile as tile
from concourse import bacc

B, S, H, R = 2, 1024, 1024, 32
N = S * H
NCORES = 8
HQ = H // 4           # h-columns per core
E = 2048              # table elements per partition (1024*256/128)
DSTS = [1440, 672, 240, 72, 24, 16, 8, 4, 4]  # per-pass dst caps
PASSES = len(DSTS)
SCANS = [E] + DSTS[:-1]                # pass k scans pass k-1's dst
DOFF = np.cumsum([0] + DSTS).tolist()  # dst offsets in packed output
IOFF = np.cumsum([0] + SCANS).tolist()  # idx offsets in packed si input
ODW = DOFF[-1]        # packed output width (2832)
SIW = IOFF[-1]        # packed int16 input width (4872)

F32 = mybir.dt.float32
BF16 = mybir.dt.bfloat16
I16 = mybir.dt.int16


def _build(reps: int = 1, compile: bool = True):
    nc = bacc.Bacc()
    x = nc.declare_dram_parameter("x", [S, H], BF16, False)
    xs = nc.declare_dram_parameter("xs", [S, HQ], BF16, False)
    wseq_t = nc.declare_dram_parameter("wseq_t", [H, R], BF16, False)
    whid_t = nc.declare_dram_parameter("whid_t", [S, R], BF16, False)
    si_all = nc.declare_dram_parameter("si_all", [128, SIW], I16, False)
    od = nc.declare_dram_parameter("od", [128, ODW], BF16, True)

    with tile.TileContext(nc) as tc, ExitStack() as ctx:
        base = ctx.enter_context(tc.tile_pool(name="base", bufs=1))
        psA = ctx.enter_context(tc.tile_pool(name="psA", bufs=2, space="PSUM"))
        psM = ctx.enter_context(tc.tile_pool(name="psM", bufs=2, space="PSUM"))
        ab = ctx.enter_context(tc.tile_pool(name="ab", bufs=2))
        mp = ctx.enter_context(tc.tile_pool(name="mp", bufs=2))
        dsp = ctx.enter_context(tc.tile_pool(name="dsp", bufs=2))

        # --- one-time loads ----------------------------------------------
        xt_sb = base.tile([128, 8, S], BF16)      # X^T[h,s]: p=h%128, k=h//128
        xs_sb = base.tile([128, 8, HQ], BF16)     # X[:, hq]: p=s%128, k=s//128
        ws_sb = base.tile([128, 8, R], BF16)      # W_seq^T rows (h-major)
        wh_sb = base.tile([128, 8, R], BF16)      # W_hid^T rows (s-major)
        si_sb = base.tile([128, SIW], I16)        # cascade idx streams

        for k in range(8):
            nc.sync.dma_start_transpose(
                out=xt_sb[:, k, :], in_=x[:, 128 * k:128 * (k + 1)]
            )
        nc.sync.dma_start(
            out=xs_sb[:],
            in_=bass.AP(tensor=xs[:].tensor, offset=0,
                        ap=[[HQ, 128], [128 * HQ, 8], [1, HQ]]),
        )
        nc.sync.dma_start(
            out=ws_sb[:],
            in_=bass.AP(tensor=wseq_t[:].tensor, offset=0,
                        ap=[[R, 128], [128 * R, 8], [1, R]]),
        )
        nc.sync.dma_start(
            out=wh_sb[:],
            in_=bass.AP(tensor=whid_t[:].tensor, offset=0,
                        ap=[[R, 128], [128 * R, 8], [1, R]]),
        )
        nc.sync.dma_start(out=si_sb[:], in_=si_all[:])

        for _ in range(reps):
            _body(nc, psA, psM, ab, mp, dsp,
                  xt_sb, xs_sb, ws_sb, wh_sb, si_sb, od)
    if compile:
        nc.compile()
    return nc


def _body(nc, psA, psM, ab, mp, dsp,
          xt_sb, xs_sb, ws_sb, wh_sb, si_sb, od):
    # --- factor matmuls: A^T [32, 1024], B^T [32, 256] -------------------
    a_t = ab.tile([R, S], BF16, tag="a_t")
    for nh in range(2):
        pa = psA.tile([R, S // 2], F32, tag="pa")
        for k in range(8):
            nc.tensor.matmul(
                out=pa[:], lhsT=ws_sb[:, k, :],
                rhs=xt_sb[:, k, nh * 512:(nh + 1) * 512],
                start=(k == 0), stop=(k == 7),
            )
        nc.vector.tensor_copy(out=a_t[:, nh * 512:(nh + 1) * 512], in_=pa[:])
    b_t = ab.tile([R, HQ], BF16, tag="b_t")
    pb = psA.tile([R, HQ], F32, tag="pb")
    for k in range(8):
        nc.tensor.matmul(
            out=pb[:], lhsT=wh_sb[:, k, :], rhs=xs_sb[:, k, :],
            start=(k == 0), stop=(k == 7),
        )
    nc.vector.tensor_copy(out=b_t[:], in_=pb[:])

    # --- M slice: [128, 2048] bf16 ---------------------------------------
    m_b = mp.tile([128, E], BF16, tag="m_b")
    for kb2 in range(4):
        pm = psM.tile([128, 2 * HQ], F32, tag="pm")
        for j in range(2):
            kb = 2 * kb2 + j
            nc.tensor.matmul(
                out=pm[:, j * HQ:(j + 1) * HQ],
                lhsT=a_t[:, kb * 128:(kb + 1) * 128], rhs=b_t[:],
                start=True, stop=True,
            )
        nc.vector.tensor_copy(out=m_b[:, kb2 * 2 * HQ:(kb2 + 1) * 2 * HQ],
                              in_=pm[:])

    # --- local_scatter cascade ------------------------------------------
    ds = dsp.tile([128, ODW], BF16, tag="ds")
    for i in range(PASSES):
        data = m_b[:] if i == 0 else ds[:, DOFF[i - 1]:DOFF[i]]
        nc.gpsimd.local_scatter(
            out_ap=ds[:, DOFF[i]:DOFF[i + 1]], data_ap=data,
            idxs_ap=si_sb[:, IOFF[i]:IOFF[i + 1]],
            channels=128, num_elems=DSTS[i], num_idxs=SCANS[i],
        )
    cut1, cut2 = 1024, 2048
    nc.scalar.dma_start(
        out=bass.AP(tensor=od[:].tensor, offset=0,
                    ap=[[ODW, 128], [1, cut1]]),
        in_=ds[:, :cut1],
    )
    nc.sync.dma_start(
        out=bass.AP(tensor=od[:].tensor, offset=cut1,
                    ap=[[ODW, 128], [1, cut2 - cut1]]),
        in_=ds[:, cut1:cut2],
    )
    nc.scalar.dma_start(
        out=bass.AP(tensor=od[:].tensor, offset=cut2,
                    ap=[[ODW, 128], [1, ODW - cut2]]),
        in_=ds[:, cut2:],
    )


# ---------------------------------------------------------------------------
# Host-side routing
# ---------------------------------------------------------------------------

def _group_slots(keys):
    """Per-group running index for a sorted int array."""
    n = len(keys)
    if n == 0:
        return np.zeros(0, np.int64)
    first = np.r_[True, keys[1:] != keys[:-1]]
    starts = np.flatnonzero(first)
    counts = np.diff(np.r_[starts, n])
    return np.arange(n) - np.repeat(starts, counts)


def _route_quarter(s, h, n_sel):
    """Route one quarter's outputs through the scatter cascade.

    Returns (si_all [128, SIW] i16, (n_ids, od flat positions) for
    device-served users, fallback n_ids)."""
    p = (s & 127).astype(np.int64)
    o = (((s >> 7) << 8) | (h & 255)).astype(np.int64)
    key = p * E + o
    order = np.argsort(key, kind="stable")
    ks = key[order]
    n_ord = n_sel[order]
    rank = _group_slots(ks)

    # element table (unique keys, key order)
    first = np.r_[True, ks[1:] != ks[:-1]]
    el_key = ks[first]
    el_cnt = np.diff(np.r_[np.flatnonzero(first), len(ks)])
    el_p = el_key // E
    el_o = el_key % E
    ne = len(el_key)
    u_el = np.cumsum(first) - 1           # user -> element index

    si_arr = np.full((128, SIW), -1, np.int16)
    el_slot = np.full((PASSES, ne), -1, np.int64)
    alive = np.ones(ne, bool)
    for k in range(PASSES):
        cand = alive & (el_cnt >= k + 1)
        idxs = np.flatnonzero(cand)
        slot = _group_slots(el_p[idxs])
        ovf = slot >= DSTS[k]
        if ovf.any():
            alive[idxs[ovf]] = False      # demote element's remaining users
            idxs, slot = idxs[~ovf], slot[~ovf]
        el_slot[k, idxs] = slot
        # device idx stream for pass k, indexed by data position j
        jpos = el_o[idxs] if k == 0 else el_slot[k - 1, idxs]
        si_arr[el_p[idxs], IOFF[k] + jpos] = slot.astype(np.int16)

    u_slot = np.where(rank < PASSES,
                      el_slot[np.minimum(rank, PASSES - 1), u_el], -1)
    okm = u_slot >= 0
    pos = (el_p[u_el[okm]] * ODW + np.asarray(DOFF)[rank[okm]]
           + u_slot[okm]).astype(np.int64)
    return si_arr, (n_ord[okm], pos), n_ord[~okm]


def prepare_in_maps(hidden_states, W_seq, W_hid, all_indices):
    x_bf = [np.ascontiguousarray(hidden_states[b].astype(ml_dtypes.bfloat16))
            for b in range(B)]
    ws_t = np.ascontiguousarray(W_seq.T.astype(ml_dtypes.bfloat16))
    wh_t = np.ascontiguousarray(W_hid.T.astype(ml_dtypes.bfloat16))

    s_idx = np.asarray(all_indices[:, 0], dtype=np.int64)
    h_idx = np.asarray(all_indices[:, 1], dtype=np.int64)
    qarr = h_idx >> 8

    routes = []
    for q in range(4):
        n_sel = np.flatnonzero(qarr == q)
        routes.append(_route_quarter(s_idx[n_sel], h_idx[n_sel], n_sel))

    in_maps = []
    for c in range(NCORES):
        b, q = c // 4, c % 4
        si_arr, _, _ = routes[q]
        in_maps.append({
            "x": x_bf[b],
            "xs": np.ascontiguousarray(x_bf[b][:, HQ * q:HQ * (q + 1)]),
            "wseq_t": ws_t,
            "whid_t": wh_t,
            "si_all": si_arr,
        })
    return in_maps, routes


def assemble(results, routes, hidden_states, W_seq, W_hid, all_indices):
    out = np.empty((B, N), dtype=np.float32)
    fb_cache = {}
    for c in range(NCORES):
        b, q = c // 4, c % 4
        _, (n_ids, pos), n_fb = routes[q]
        buf = np.asarray(results[c]["od"], np.float32).reshape(-1)
        out[b, n_ids] = buf[pos]
        if len(n_fb):
            # host fallback: elements that overflowed a dst cap or have
            # multiplicity > PASSES (never for uniform random indices)
            if b not in fb_cache:
                X = np.asarray(hidden_states[b], np.float32)
                A = X @ np.asarray(W_seq, np.float32).T        # [S, R]
                Bm = X.T @ np.asarray(W_hid, np.float32).T     # [H, R]
                fb_cache[b] = (A, Bm)
            A, Bm = fb_cache[b]
            si = np.asarray(all_indices[n_fb, 0], np.int64)
            hi = np.asarray(all_indices[n_fb, 1], np.int64)
            out[b, n_fb] = np.einsum("nr,nr->n", A[si], Bm[hi])
    return out.reshape(B, S, H)


# ---------------------------------------------------------------------------
# Runner (trace/compile SPMD executable once, reuse)
# ---------------------------------------------------------------------------

_nc_cache_by_reps = {}


def _get_nc(reps: int = 1):
    nc = _nc_cache_by_reps.get(reps)
    if nc is None:
        nc = _nc_cache_by_reps[reps] = _build(reps)
    return nc


class _Runner:
    """Trace/compile the SPMD executable once; reuse across calls."""

    def __init__(self, nc, donate=True):
        import jax
        from jax.experimental.shard_map import shard_map
        from jax.sharding import Mesh, PartitionSpec
        import concourse.bass2jax as b2j

        b2j.install_neuronx_cc_hook()
        self.nc = nc
        part_name = (nc.partition_id_tensor.name
                     if nc.partition_id_tensor else None)
        in_names, out_names, out_avals = [], [], []
        zero_outs = []
        for alloc in nc.m.functions[0].allocations:
            if not isinstance(alloc, mybir.MemoryLocationSet):
                continue
            name = alloc.memorylocations[0].name
            if alloc.kind == "ExternalInput":
                if name != part_name:
                    in_names.append(name)
            elif alloc.kind == "ExternalOutput":
                out_names.append(name)
                shape = tuple(alloc.tensor_shape)
                dtype = mybir.dt.np(alloc.dtype)
                out_avals.append(jax.core.ShapedArray(shape, dtype))
                zero_outs.append(np.zeros(shape, dtype))
        self.in_names = list(in_names)
        self.out_names = out_names
        self.zero_outs = zero_outs
        n_params = len(in_names)
        n_outs = len(out_names)
        all_in_names = in_names + out_names
        if part_name is not None:
            all_in_names = all_in_names + [part_name]
        donate_nums = (tuple(range(n_params, n_params + n_outs))
                       if donate else ())

        def _body_fn(*args):
            operands = list(args)
            if part_name is not None:
                operands.append(b2j.partition_id_tensor())
            outs = b2j._bass_exec_p.bind(
                *operands,
                out_avals=tuple(out_avals),
                in_names=tuple(all_in_names),
                out_names=tuple(out_names),
                lowering_input_output_aliases=(),
                sim_require_finite=True,
                sim_require_nnan=True,
                nc=nc,
            )
            return tuple(outs)

        devices = jax.devices()[:NCORES]
        mesh = Mesh(np.asarray(devices), ("core",))
        self.fn = jax.jit(
            shard_map(
                _body_fn, mesh=mesh,
                in_specs=(PartitionSpec("core"),) * (n_params + n_outs),
                out_specs=(PartitionSpec("core"),) * n_outs,
                check_rep=False,
            ),
            donate_argnums=donate_nums,
            keep_unused=True,
        )
        self.mesh = mesh

    def __call__(self, in_maps):
        concat_in = [
            np.concatenate([np.asarray(m[name]) for m in in_maps], axis=0)
            for name in self.in_names
        ]
        concat_zeros = [
            np.zeros((NCORES * z.shape[0], *z.shape[1:]), z.dtype)
            for z in self.zero_outs
        ]
        out_arrs = self.fn(*concat_in, *concat_zeros)
        return [
            {
                name: np.asarray(out_arrs[i]).reshape(
                    NCORES, *self.zero_outs[i].shape)[c]
                for i, name in enumerate(self.out_names)
            }
            for c in range(NCORES)
        ]


_runner_cache = {}


def _get_runner(reps: int = 1):
    r = _runner_cache.get(reps)
    if r is None:
        r = _runner_cache[reps] = _Runner(_get_nc(reps))
    return r


def kernel(hidden_states, W_seq, W_hid, all_indices):
    hidden_states = np.asarray(hidden_states)
    W_seq = np.asarray(W_seq)
    W_hid = np.asarray(W_hid)
    all_indices = np.asarray(all_indices)

    runner = _get_runner()
    in_maps, routes = prepare_in_maps(hidden_states, W_seq, W_hid, all_indices)
    results = runner(in_maps)
    return assemble(results, routes, hidden_states, W_seq, W_hid, all_indices)


# revision 4
# speedup vs baseline: 3.4354x; 1.0368x over previous
"""Trainium2 Bass kernel for nn_CPCircuitLayer (embedding_lookup), v4.

Math: out[b, n] = dot(A[b, idx_s[n]], Bm[b, idx_h[n]]) = M_b[idx_s[n], idx_h[n]]
where M_b = A_b @ Bm_b^T is a [S, H] table, A = X W_seq^T, Bm^T = W_hid X.

Sharding (8 cores, no collectives): core c handles batch b = c//4 and the
h-slice q = c%4 (columns [256q, 256q+256)). It builds only its M slice
[1024, 256] laid out as [128 partitions, 2048]: partition p = s%128,
offset o = (s//128)*256 + h%256. Each output n is routed to the core
owning its table element.

Per-core device pipeline:
  1. PE matmuls (bf16 in, f32 psum): A^T [32,1024], B^T [32,256], then
     M = A @ B^T -> sbuf bf16 [128, 2048].
  2. local_scatter cascade: pass k serves the rank-k user of every table
     element (host-prepped per-partition dst slots, in key order). Pass 0
     scans the table; pass k>=1 scans pass (k-1)'s dst buffer, which is
     exactly the compacted list of elements with >= k users. Scan lengths
     shrink geometrically, so 12 passes (multiplicity <= 12) cost barely
     more than 3. The packed dst buffer IS the output: one DMA to DRAM
     (split across queues), host unpermutes.
  3. Outputs whose element overflowed a dst cap or has multiplicity > 12
     (never happens for uniform random indices) are computed on host.
"""

import numpy as np
import ml_dtypes
from contextlib import ExitStack

import concourse.bass as bass
import concourse.mybir as mybir
import concourse.tile as tile
from concourse import bacc

B, S, H, R = 2, 1024, 1024, 32
N = S * H
NCORES = 8
HQ = H // 4           # h-columns per core
E = 2048              # table elements per partition (1024*256/128)
DSTS = [1440, 672, 240, 72, 24, 16, 8, 4, 4]  # per-pass dst caps
PASSES = len(DSTS)
SCANS = [E] + DSTS[:-1]                # pass k scans pass k-1's dst
DOFF = np.cumsum([0] + DSTS).tolist()  # dst offsets in packed output
IOFF = np.cumsum([0] + SCANS).tolist()  # idx offsets in packed si input
ODW = DOFF[-1]        # packed output width (2832)
SIW = IOFF[-1]        # packed int16 input width (4872)

F32 = mybir.dt.float32
BF16 = mybir.dt.bfloat16
I16 = mybir.dt.int16


def _build(reps: int = 1, compile: bool = True):
    nc = bacc.Bacc()
    x = nc.declare_dram_parameter("x", [S, H], BF16, False)
    xs = nc.declare_dram_parameter("xs", [S, HQ], BF16, False)
    wseq_t = nc.declare_dram_parameter("wseq_t", [H, R], BF16, False)
    whid_t = nc.declare_dram_parameter("whid_t", [S, R], BF16, False)
    si_all = nc.declare_dram_parameter("si_all", [128, SIW], I16, False)
    od = nc.declare_dram_parameter("od", [128, ODW], BF16, True)

    with tile.TileContext(nc) as tc, ExitStack() as ctx:
        base = ctx.enter_context(tc.tile_pool(name="base", bufs=1))
        psA = ctx.enter_context(tc.tile_pool(name="psA", bufs=2, space="PSUM"))
        psM = ctx.enter_context(tc.tile_pool(name="psM", bufs=2, space="PSUM"))
        ab = ctx.enter_context(tc.tile_pool(name="ab", bufs=2))
        mp = ctx.enter_context(tc.tile_pool(name="mp", bufs=2))
        dsp = ctx.enter_context(tc.tile_pool(name="dsp", bufs=2))

        # --- one-time loads ----------------------------------------------
        xt_sb = base.tile([128, 8, S], BF16)      # X^T[h,s]: p=h%128, k=h//128
        xs_sb = base.tile([128, 8, HQ], BF16)     # X[:, hq]: p=s%128, k=s//128
        ws_sb = base.tile([128, 8, R], BF16)      # W_seq^T rows (h-major)
        wh_sb = base.tile([128, 8, R], BF16)      # W_hid^T rows (s-major)
        si_sb = base.tile([128, SIW], I16)        # cascade idx streams

        for k in range(8):
            nc.sync.dma_start_transpose(
                out=xt_sb[:, k, :], in_=x[:, 128 * k:128 * (k + 1)]
            )
        nc.sync.dma_start(
            out=xs_sb[:],
            in_=bass.AP(tensor=xs[:].tensor, offset=0,
                        ap=[[HQ, 128], [128 * HQ, 8], [1, HQ]]),
        )
        nc.sync.dma_start(
            out=ws_sb[:],
            in_=bass.AP(tensor=wseq_t[:].tensor, offset=0,
                        ap=[[R, 128], [128 * R, 8], [1, R]]),
        )
        nc.sync.dma_start(
            out=wh_sb[:],
            in_=bass.AP(tensor=whid_t[:].tensor, offset=0,
                        ap=[[R, 128], [128 * R, 8], [1, R]]),
        )
        nc.sync.dma_start(out=si_sb[:], in_=si_all[:])

        for _ in range(reps):
            _body(nc, psA, psM, ab, mp, dsp,
                  xt_sb, xs_sb, ws_sb, wh_sb, si_sb, od)
    if compile:
        nc.compile()
    return nc


def _body(nc, psA, psM, ab, mp, dsp,
          xt_sb, xs_sb, ws_sb, wh_sb, si_sb, od):
    # --- factor matmuls: A^T [32, 1024], B^T [32, 256] -------------------
    a_t = ab.tile([R, S], BF16, tag="a_t")
    for nh in range(2):
        pa = psA.tile([R, S // 2], F32, tag="pa")
        for k in range(8):
            nc.tensor.matmul(
                out=pa[:], lhsT=ws_sb[:, k, :],
                rhs=xt_sb[:, k, nh * 512:(nh + 1) * 512],
                start=(k == 0), stop=(k == 7),
            )
        nc.vector.tensor_copy(out=a_t[:, nh * 512:(nh + 1) * 512], in_=pa[:])
    b_t = ab.tile([R, HQ], BF16, tag="b_t")
    pb = psA.tile([R, HQ], F32, tag="pb")
    for k in range(8):
        nc.tensor.matmul(
            out=pb[:], lhsT=wh_sb[:, k, :], rhs=xs_sb[:, k, :],
            start=(k == 0), stop=(k == 7),
        )
    nc.scalar.copy(out=b_t[:], in_=pb[:])

    # --- M slice: [128, 2048] bf16 ---------------------------------------
    m_b = mp.tile([128, E], BF16, tag="m_b")
    for kb4 in range(2):
        pm = psM.tile([128, 4 * HQ], F32, tag="pm")
        for j in range(4):
            kb = 4 * kb4 + j
            nc.tensor.matmul(
                out=pm[:, j * HQ:(j + 1) * HQ],
                lhsT=a_t[:, kb * 128:(kb + 1) * 128], rhs=b_t[:],
                start=True, stop=True,
            )
        ceng = nc.vector.tensor_copy if kb4 == 0 else (
            lambda out, in_: nc.scalar.copy(out=out, in_=in_))
        ceng(out=m_b[:, kb4 * 4 * HQ:(kb4 + 1) * 4 * HQ], in_=pm[:])

    # --- local_scatter cascade ------------------------------------------
    ds = dsp.tile([128, ODW], BF16, tag="ds")
    for i in range(PASSES):
        data = m_b[:] if i == 0 else ds[:, DOFF[i - 1]:DOFF[i]]
        nc.gpsimd.local_scatter(
            out_ap=ds[:, DOFF[i]:DOFF[i + 1]], data_ap=data,
            idxs_ap=si_sb[:, IOFF[i]:IOFF[i + 1]],
            channels=128, num_elems=DSTS[i], num_idxs=SCANS[i],
        )
    cut1, cut2 = 1024, 2048
    nc.scalar.dma_start(
        out=bass.AP(tensor=od[:].tensor, offset=0,
                    ap=[[ODW, 128], [1, cut1]]),
        in_=ds[:, :cut1],
    )
    nc.sync.dma_start(
        out=bass.AP(tensor=od[:].tensor, offset=cut1,
                    ap=[[ODW, 128], [1, cut2 - cut1]]),
        in_=ds[:, cut1:cut2],
    )
    nc.scalar.dma_start(
        out=bass.AP(tensor=od[:].tensor, offset=cut2,
                    ap=[[ODW, 128], [1, ODW - cut2]]),
        in_=ds[:, cut2:],
    )


# ---------------------------------------------------------------------------
# Host-side routing
# ---------------------------------------------------------------------------

def _group_slots(keys):
    """Per-group running index for a sorted int array."""
    n = len(keys)
    if n == 0:
        return np.zeros(0, np.int64)
    first = np.r_[True, keys[1:] != keys[:-1]]
    starts = np.flatnonzero(first)
    counts = np.diff(np.r_[starts, n])
    return np.arange(n) - np.repeat(starts, counts)


def _route_quarter(s, h, n_sel):
    """Route one quarter's outputs through the scatter cascade.

    Returns (si_all [128, SIW] i16, (n_ids, od flat positions) for
    device-served users, fallback n_ids)."""
    p = (s & 127).astype(np.int64)
    o = (((s >> 7) << 8) | (h & 255)).astype(np.int64)
    key = p * E + o
    order = np.argsort(key, kind="stable")
    ks = key[order]
    n_ord = n_sel[order]
    rank = _group_slots(ks)

    # element table (unique keys, key order)
    first = np.r_[True, ks[1:] != ks[:-1]]
    el_key = ks[first]
    el_cnt = np.diff(np.r_[np.flatnonzero(first), len(ks)])
    el_p = el_key // E
    el_o = el_key % E
    ne = len(el_key)
    u_el = np.cumsum(first) - 1           # user -> element index

    si_arr = np.full((128, SIW), -1, np.int16)
    el_slot = np.full((PASSES, ne), -1, np.int64)
    alive = np.ones(ne, bool)
    for k in range(PASSES):
        cand = alive & (el_cnt >= k + 1)
        idxs = np.flatnonzero(cand)
        slot = _group_slots(el_p[idxs])
        ovf = slot >= DSTS[k]
        if ovf.any():
            alive[idxs[ovf]] = False      # demote element's remaining users
            idxs, slot = idxs[~ovf], slot[~ovf]
        el_slot[k, idxs] = slot
        # device idx stream for pass k, indexed by data position j
        jpos = el_o[idxs] if k == 0 else el_slot[k - 1, idxs]
        si_arr[el_p[idxs], IOFF[k] + jpos] = slot.astype(np.int16)

    u_slot = np.where(rank < PASSES,
                      el_slot[np.minimum(rank, PASSES - 1), u_el], -1)
    okm = u_slot >= 0
    pos = (el_p[u_el[okm]] * ODW + np.asarray(DOFF)[rank[okm]]
           + u_slot[okm]).astype(np.int64)
    return si_arr, (n_ord[okm], pos), n_ord[~okm]


def prepare_in_maps(hidden_states, W_seq, W_hid, all_indices):
    x_bf = [np.ascontiguousarray(hidden_states[b].astype(ml_dtypes.bfloat16))
            for b in range(B)]
    ws_t = np.ascontiguousarray(W_seq.T.astype(ml_dtypes.bfloat16))
    wh_t = np.ascontiguousarray(W_hid.T.astype(ml_dtypes.bfloat16))

    s_idx = np.asarray(all_indices[:, 0], dtype=np.int64)
    h_idx = np.asarray(all_indices[:, 1], dtype=np.int64)
    qarr = h_idx >> 8

    routes = []
    for q in range(4):
        n_sel = np.flatnonzero(qarr == q)
        routes.append(_route_quarter(s_idx[n_sel], h_idx[n_sel], n_sel))

    in_maps = []
    for c in range(NCORES):
        b, q = c // 4, c % 4
        si_arr, _, _ = routes[q]
        in_maps.append({
            "x": x_bf[b],
            "xs": np.ascontiguousarray(x_bf[b][:, HQ * q:HQ * (q + 1)]),
            "wseq_t": ws_t,
            "whid_t": wh_t,
            "si_all": si_arr,
        })
    return in_maps, routes


def assemble(results, routes, hidden_states, W_seq, W_hid, all_indices):
    out = np.empty((B, N), dtype=np.float32)
    fb_cache = {}
    for c in range(NCORES):
        b, q = c // 4, c % 4
        _, (n_ids, pos), n_fb = routes[q]
        buf = np.asarray(results[c]["od"], np.float32).reshape(-1)
        out[b, n_ids] = buf[pos]
        if len(n_fb):
            # host fallback: elements that overflowed a dst cap or have
            # multiplicity > PASSES (never for uniform random indices)
            if b not in fb_cache:
                X = np.asarray(hidden_states[b], np.float32)
                A = X @ np.asarray(W_seq, np.float32).T        # [S, R]
                Bm = X.T @ np.asarray(W_hid, np.float32).T     # [H, R]
                fb_cache[b] = (A, Bm)
            A, Bm = fb_cache[b]
            si = np.asarray(all_indices[n_fb, 0], np.int64)
            hi = np.asarray(all_indices[n_fb, 1], np.int64)
            out[b, n_fb] = np.einsum("nr,nr->n", A[si], Bm[hi])
    return out.reshape(B, S, H)


# ---------------------------------------------------------------------------
# Runner (trace/compile SPMD executable once, reuse)
# ---------------------------------------------------------------------------

_nc_cache_by_reps = {}


def _get_nc(reps: int = 1):
    nc = _nc_cache_by_reps.get(reps)
    if nc is None:
        nc = _nc_cache_by_reps[reps] = _build(reps)
    return nc


class _Runner:
    """Trace/compile the SPMD executable once; reuse across calls."""

    def __init__(self, nc, donate=True):
        import jax
        from jax.experimental.shard_map import shard_map
        from jax.sharding import Mesh, PartitionSpec
        import concourse.bass2jax as b2j

        b2j.install_neuronx_cc_hook()
        self.nc = nc
        part_name = (nc.partition_id_tensor.name
                     if nc.partition_id_tensor else None)
        in_names, out_names, out_avals = [], [], []
        zero_outs = []
        for alloc in nc.m.functions[0].allocations:
            if not isinstance(alloc, mybir.MemoryLocationSet):
                continue
            name = alloc.memorylocations[0].name
            if alloc.kind == "ExternalInput":
                if name != part_name:
                    in_names.append(name)
            elif alloc.kind == "ExternalOutput":
                out_names.append(name)
                shape = tuple(alloc.tensor_shape)
                dtype = mybir.dt.np(alloc.dtype)
                out_avals.append(jax.core.ShapedArray(shape, dtype))
                zero_outs.append(np.zeros(shape, dtype))
        self.in_names = list(in_names)
        self.out_names = out_names
        self.zero_outs = zero_outs
        n_params = len(in_names)
        n_outs = len(out_names)
        all_in_names = in_names + out_names
        if part_name is not None:
            all_in_names = all_in_names + [part_name]
        donate_nums = (tuple(range(n_params, n_params + n_outs))
                       if donate else ())

        def _body_fn(*args):
            operands = list(args)
            if part_name is not None:
                operands.append(b2j.partition_id_tensor())
            outs = b2j._bass_exec_p.bind(
                *operands,
                out_avals=tuple(out_avals),
                in_names=tuple(all_in_names),
                out_names=tuple(out_names),
                lowering_input_output_aliases=(),
                sim_require_finite=True,
                sim_require_nnan=True,
                nc=nc,
            )
            return tuple(outs)

        devices = jax.devices()[:NCORES]
        mesh = Mesh(np.asarray(devices), ("core",))
        self.fn = jax.jit(
            shard_map(
                _body_fn, mesh=mesh,
                in_specs=(PartitionSpec("core"),) * (n_params + n_outs),
                out_specs=(PartitionSpec("core"),) * n_outs,
                check_rep=False,
            ),
            donate_argnums=donate_nums,
            keep_unused=True,
        )
        self.mesh = mesh

    def __call__(self, in_maps):
        concat_in = [
            np.concatenate([np.asarray(m[name]) for m in in_maps], axis=0)
            for name in self.in_names
        ]
        concat_zeros = [
            np.zeros((NCORES * z.shape[0], *z.shape[1:]), z.dtype)
            for z in self.zero_outs
        ]
        out_arrs = self.fn(*concat_in, *concat_zeros)
        return [
            {
                name: np.asarray(out_arrs[i]).reshape(
                    NCORES, *self.zero_outs[i].shape)[c]
                for i, name in enumerate(self.out_names)
            }
            for c in range(NCORES)
        ]


_runner_cache = {}


def _get_runner(reps: int = 1):
    r = _runner_cache.get(reps)
    if r is None:
        r = _runner_cache[reps] = _Runner(_get_nc(reps))
    return r


def kernel(hidden_states, W_seq, W_hid, all_indices):
    hidden_states = np.asarray(hidden_states)
    W_seq = np.asarray(W_seq)
    W_hid = np.asarray(W_hid)
    all_indices = np.asarray(all_indices)

    runner = _get_runner()
    in_maps, routes = prepare_in_maps(hidden_states, W_seq, W_hid, all_indices)
    results = runner(in_maps)
    return assemble(results, routes, hidden_states, W_seq, W_hid, all_indices)
